# revision 1
# baseline (speedup 1.0000x reference)
"""DCRNN seq2seq (encoder/decoder DCGRU, K=3 Chebyshev diffusion) on 8 NeuronCores.

Sharding: data-parallel over batch (8 batch elements per core); weights and the
200x200 support replicated; no collectives.

Per-core layout (v2 — batched transposes + (n-chunk x batch)-batched matmuls):
  - f-major state per layer is a PAIR of tiles  Ha [64u, 8b, 128n], Hb [64u, 8b, 80n]
    (n split 0:128 / 128:208, cols 200:208 zero-pad).  This makes the f-major ->
    node-major conversion TWO xbar DMA transposes per quantity:
      in [64, 1024] -> out [128, 8b, 64u]   (dst = slot of the node-major tile)
      in [64,  640] -> out [ 80, 8b, 64u]
    (out[p,e,c] = in[c, e*P+p], matching the b-major-outer source layout).
  - Diffusion (contract over nodes): per-b lhsT from the node-major slot tiles,
    rhs = [S1 | S2] -> psum [feat, 400], evacuated bf16 to per-b diffused tiles.
    S2 = 2*S@S - I precomputed host-side.
  - Gate/candidate matmuls (contract features) batch ALL 8 b per n-chunk:
    F = nw*8 <= 512 per matmul (n-chunks 64/64/64/8); rhs APs are strided
    (k b n -> k n b) views of the f-major / diffused tiles; psum [P, nw, 8b].
  - ONE sigmoid per n-chunk computes r and u together (128 partitions) into an
    n-major RU tile [128, 208, 8]; tanh likewise into CFM [64, 208, 8].
  - GRU elementwise on VectorE per state-pair tile; in-place update of Ha/Hb.
  - Decoder projection: lhsT = [h3; ones] per-b slices, rhs = [proj_W; proj_b];
    decoder layer-0 k=0 x-term algebraically fused via dWfg/dWfc = Wp @ W0x_k0.

All matmul operands bf16 (fp32 psum accumulate).
"""

import numpy as np
import ml_dtypes

import concourse.bass as bass
import concourse.tile as tile
from concourse import bacc, mybir
from concourse.bass_utils import run_bass_kernel_spmd

BF = ml_dtypes.bfloat16
F32 = np.float32

N = 200
U = 64
L = 4
T = 12
B = 64
NCORES = 8
BL = B // NCORES
M0, M1 = 128, 72
NPAD = 208
NB = 128  # n width of the 'b' half-tile (covers n 128:256; valid to 200)
NCH = [(0, 64), (64, 64), (128, 64), (192, 8)]

dt = mybir.dt
AF = mybir.ActivationFunctionType

_CACHE = {}
DBG = False


def _r3(ap):
    # [K, b, n] view -> [K, n, b] iteration order (for matmul rhs / psum order)
    return ap.rearrange("k b n -> k n b")


def _rn(ap):
    # [K, n, b] view -> [K, b, n] iteration order (for elementwise vs b-major)
    return ap.rearrange("k n b -> k b n")


def _build(enc_T=T, dec_T=T):
    nc = bacc.Bacc()

    d = {}

    def din(name, shape, dtype=dt.bfloat16):
        d[name] = nc.dram_tensor(name, shape, dtype, kind='ExternalInput')

    din('SS0', [M0, 400])
    din('SS1', [M1, 400])
    din('Wp', [U + 1, 200])
    for p in ('e', 'd'):
        din(p + 'g0x', [200, 3, 128])
        din(p + 'g0h', [64, 3, 128])
        din(p + 'c0x', [200, 3, 64])
        din(p + 'c0h', [64, 3, 64])
        din(p + 'gL', [128, 3, 3, 128])
        din(p + 'gLh', [64, 3, 128])
        din(p + 'cLk0x', [64, 3, 64])
        din(p + 'cLh', [64, 3, 64])
        din(p + 'cLx', [64, 3, 2, 64])
        din(p + 'cLrh', [64, 3, 2, 64])
        din(p + 'bg', [128, 4], dt.float32)
        din(p + 'bc', [64, 4], dt.float32)
    din('dWfg', [64, 128])
    din('dWfc', [64, 64])
    din('xTe', [enc_T, 2, M0, BL, 200])
    din('xfme', [enc_T, 2, M0, BL, 200])
    d['onm'] = nc.dram_tensor('onm', [max(dec_T, 1), 200, BL, 200], dt.float32,
                              kind='ExternalOutput')
    dbg = {}

    def dbg_out(name, shape, dtype=dt.bfloat16):
        dbg[name] = nc.dram_tensor('dbg_' + name, shape, dtype,
                                   kind='ExternalOutput')
        return dbg[name]
    if DBG:
        for nm, sh in (('Xgh', [64, BL, 400]), ('Xga', [M0, BL, 400]),
                       ('Xgb', [M1, BL, 400]), ('RFM', [64, BL, 128]),
                       ('UFM', [64, BL, 128]), ('CFM', [64, BL, 128]),
                       ('RHa', [64, BL, 128]), ('RHb', [64, BL, NB]),
                       ('Xrh', [64, BL, 400]), ('HA0', [64, BL, 128]),
                       ('HB0', [64, BL, NB]), ('RHT0', [M0, 4, BL, 64]),
                       ('RHT1', [NB, 4, BL, 64]), ('HTW0', [M0, 4, BL, 64]),
                       ('HTW1', [NB, 4, BL, 64])):
            dbg_out(nm, sh)

    with tile.TileContext(nc) as tc:
        with (
            tc.tile_pool(name='const', bufs=1) as cp,
            tc.tile_pool(name='state', bufs=1) as sp,
            tc.tile_pool(name='work3', bufs=3) as wp3,
            tc.tile_pool(name='work', bufs=3) as wp,
            tc.tile_pool(name='work2', bufs=2) as wp2,
            tc.tile_pool(name='xin', bufs=2) as xp,
            tc.tile_pool(name='dps', bufs=3, space='PSUM') as diffps,
            tc.tile_pool(name='ops', bufs=4, space='PSUM') as gps,
            tc.tile_pool(name='opsn', bufs=1, space='PSUM') as gpsn,
        ):
            # ---- load constants / weights ----
            CT = {}
            for name, t_ in d.items():
                if name in ('onm', 'xTe', 'xfme'):
                    continue
                shape = list(t_.shape)
                if shape[0] == 200:  # split node-feature-major weights
                    CT[name + '@a'] = cp.tile([M0] + shape[1:], t_.dtype, name='t' + name + 'a')
                    CT[name + '@b'] = cp.tile([M1] + shape[1:], t_.dtype, name='t' + name + 'b')
                    nc.sync.dma_start(out=CT[name + '@a'], in_=t_[0:M0])
                    nc.sync.dma_start(out=CT[name + '@b'], in_=t_[M0:200])
                else:
                    CT[name] = cp.tile(shape, t_.dtype, name='t' + name)
                    nc.sync.dma_start(out=CT[name], in_=t_[:])
            SS = [CT['SS0'], CT['SS1']]
            Wp = CT['Wp']

            # ---- state ----
            HA = [sp.tile([64, BL, 128], dt.bfloat16, name=f'HA{i}') for i in range(3)]
            HB = [sp.tile([64, BL, NB], dt.bfloat16, name=f'HB{i}') for i in range(3)]
            HA.append(sp.tile([65, BL, 128], dt.bfloat16, name='HA3'))
            HB.append(sp.tile([65, BL, NB], dt.bfloat16, name='HB3'))
            # node-major storage, slot-OUTER so every slot is a contiguous xbar
            # transpose destination and b-adjacent pairs form one P=128 lhsT:
            #   HLT: h per layer;  RHT: r*h per layer
            HLT0 = sp.tile([M0, 4, BL, 64], dt.bfloat16, name='HLT0')
            HLT1 = sp.tile([NB, 4, BL, 64], dt.bfloat16, name='HLT1')
            RHT0 = sp.tile([M0, 4, BL, 64], dt.bfloat16, name='RHT0')
            RHT1 = sp.tile([NB, 4, BL, 64], dt.bfloat16, name='RHT1')

            for t_ in HA + HB + [HLT0, HLT1, RHT0, RHT1]:
                nc.vector.memset(t_[:], 0.0)
            nc.vector.memset(HA[3][64:65], 1.0)
            nc.vector.memset(HB[3][64:65], 1.0)

            def evac(i, dst, src):
                if i % 2 == 0:
                    nc.scalar.copy(dst, src)
                else:
                    nc.vector.tensor_copy(dst, src)

            def diffuse(rows, lhs, dst_ap, i):
                """psum[0:rows, 0:400] = [lhs.T @ S1 | lhs.T @ S2], evacuated
                (bf16) to dst_ap. lhs = per-m-chunk lhsT APs."""
                ps = diffps.tile([M0, 400], dt.float32, name='dps', tag='dps')
                nc.tensor.matmul(ps[0:rows, :], lhs[0], SS[0][:], start=True, stop=False)
                nc.tensor.matmul(ps[0:rows, :], lhs[1], SS[1][:], start=False, stop=True)
                evac(i, dst_ap, ps[0:rows, :])

            def diffuse_half(slot, dst, rows, b):
                """Diffuse the b:b+2 pair of h slot `slot` into dst[rows, b(+1), :]
                (rows = the 64-row half of the diffused-gates tile this half
                feeds).  psum rows 0:64 -> b, 64:128 -> b+1."""
                ps = diffps.tile([M0, 400], dt.float32, name='dps', tag='dps')
                nc.tensor.matmul(ps[:], HLT0[:, slot, b:b + 2, :], SS[0][:], start=True, stop=False)
                nc.tensor.matmul(ps[:], HLT1[0:M1, slot, b:b + 2, :], SS[1][:], start=False, stop=True)
                r0 = rows.start
                if r0 == 0:
                    nc.vector.tensor_copy(dst[0:64, b, :], ps[0:64, :])
                    nc.scalar.copy(dst[0:64, b + 1, :], ps[64:128, :])
                else:
                    nc.scalar.copy(dst[64:128, b, :], ps[0:64, :])
                    nc.vector.tensor_copy(dst[64:128, b + 1, :], ps[64:128, :])

            def diffuse_pair(lhs, dst, b):
                """Diffuse a b-adjacent pair of 64-wide node-major quantities in
                one P=128 matmul group; psum rows 0:64 -> b, 64:128 -> b+1."""
                ps = diffps.tile([M0, 400], dt.float32, name='dps', tag='dps')
                nc.tensor.matmul(ps[:], lhs[0], SS[0][:], start=True, stop=False)
                nc.tensor.matmul(ps[:], lhs[1], SS[1][:], start=False, stop=True)
                nc.vector.tensor_copy(dst[0:64, b, :], ps[0:64, :])
                nc.scalar.copy(dst[0:64, b + 1, :], ps[64:128, :])

            def fm(pa, pb, ci, rows=64):
                """f-major state rhs for n-chunk ci: [rows, 8b, nw] (b-major)."""
                n0, nw = NCH[ci]
                if ci < 2:
                    return pa[0:rows, :, n0:n0 + nw]
                return pb[0:rows, :, n0 - 128:n0 - 128 + nw]

            def pr(pa, pb, ci):
                """(pair-tile, local n-slice) for n-chunk ci."""
                n0, nw = NCH[ci]
                if ci < 2:
                    return pa, slice(n0, n0 + nw)
                return pb, slice(n0 - 128, n0 - 128 + nw)

            def transpose_rh(l, qa, qb):
                nc.sync.dma_start_transpose(RHT0[:, l, :, :], qa)
                nc.sync.dma_start_transpose(RHT1[:, l, :, :], qb)

            def transpose_h(l, qa, qb):
                nc.sync.dma_start_transpose(HLT0[:, l, :, :], qa)
                nc.sync.dma_start_transpose(HLT1[:, l, :, :], qb)

            cellno = [0]

            def rh_and_cand_tail(p, l, RFMa, RFMb, UFMa, UFMb, cand_terms):
                """r*h -> node-major rh slot -> diffuse -> cand matmuls -> tanh
                -> GRU tail -> h transposes."""
                RHa = wp2.tile([64, BL, 128], dt.bfloat16, name='RHa', tag='RHa')
                RHb = wp2.tile([64, BL, NB], dt.bfloat16, name='RHb', tag='RHb')
                nc.vector.tensor_mul(RHa[:], RFMa[:], HA[l][0:64])
                nc.vector.tensor_mul(RHb[0:64, :, 0:72], RFMb[0:64, :, 0:72],
                                     HB[l][0:64, :, 0:72])
                nc.vector.memset(RHb[0:64, :, 72:NB], 0.0)
                first_cell = DBG and cellno[0] == 0
                if first_cell:
                    nc.sync.dma_start(out=dbg['RFM'][:], in_=RFMa[:])
                    nc.sync.dma_start(out=dbg['UFM'][:], in_=UFMa[:])
                    nc.sync.dma_start(out=dbg['RHa'][:], in_=RHa[:])
                    nc.sync.dma_start(out=dbg['RHb'][:], in_=RHb[:])
                transpose_rh(l, RHa[:], RHb[:])
                Xrh = wp2.tile([64, BL, 400], dt.bfloat16, name='Xrh', tag='Xh')
                for b in range(0, BL, 2):
                    diffuse_pair([RHT0[:, l, b:b + 2, :], RHT1[0:M1, l, b:b + 2, :]],
                                 Xrh, b)
                CFMa = wp2.tile([64, BL, 128], dt.bfloat16, name='CFMa', tag='CFMa')
                CFMb = wp2.tile([64, BL, NB], dt.bfloat16, name='CFMb', tag='CFMb')
                bc = CT[p + 'bc'][:, l:l + 1]
                # col-tiled pairs: (c0,c1) share a psum bank on column groups
                # (0,0)/(0,64); (c2,c3) likewise but c3 gets its own narrow tile.
                for pi, (cx, cy) in enumerate(((0, 1), (2, 3))):
                    tx = cand_terms(cx, Xrh, RHa, RHb)
                    ty = cand_terms(cy, Xrh, RHa, RHb)
                    nwx, nwy = NCH[cx][1], NCH[cy][1]
                    psx = gps.tile([M0, BL, nwx], dt.float32, name='ops', tag='ops')
                    if nwy == nwx:
                        psy = gps.tile([M0, BL, nwy], dt.float32, name='ops', tag='ops')
                    else:
                        psy = gpsn.tile([M0, BL, nwy], dt.float32, name='opsn', tag='opsn')
                    nterm = len(tx)
                    for j, ((wx, rx), (wy, ry)) in enumerate(zip(tx, ty)):
                        first, last = j == 0, j == nterm - 1
                        nc.tensor.matmul(psx[0:64, :, :], wx, rx,
                                         start=first, stop=last,
                                         tile_position=(0, 0))
                        nc.tensor.matmul(psy[64:128, :, :], wy, ry,
                                         start=first, stop=last,
                                         tile_position=(0, 64))
                    ctx_, slx = pr(CFMa, CFMb, cx)
                    cty_, sly = pr(CFMa, CFMb, cy)
                    nc.scalar.activation(ctx_[0:64, :, slx], psx[0:64, :, :],
                                         AF.Tanh, bias=bc, scale=1.0)
                    nc.scalar.activation(cty_[0:64, :, sly], psy[64:128, :, :],
                                         AF.Tanh, bias=bc, scale=1.0)
                # ---- GRU tail:  h = c + u*(h - c)  (in place, per pair tile) ----
                TMPa = wp2.tile([64, BL, 128], dt.bfloat16, name='TMPa', tag='TMPa')
                TMPb = wp2.tile([64, BL, NB], dt.bfloat16, name='TMPb', tag='TMPb')
                ha = HA[l][0:64]
                hb = HB[l][0:64, :, 0:72]
                ca = CFMa[:]
                cb = CFMb[0:64, :, 0:72]
                nc.vector.tensor_sub(TMPa[:], ha, ca)
                nc.vector.tensor_mul(TMPa[:], UFMa[:], TMPa[:])
                nc.vector.tensor_add(ha, ca, TMPa[:])
                tb = TMPb[0:64, :, 0:72]
                nc.vector.tensor_sub(tb, hb, cb)
                nc.vector.tensor_mul(tb, UFMb[0:64, :, 0:72], tb)
                nc.vector.tensor_add(hb, cb, tb)
                transpose_h(l, HA[l][0:64], HB[l][0:64])
                if first_cell:
                    nc.sync.dma_start(out=dbg['Xrh'][:], in_=Xrh[:])
                    nc.sync.dma_start(out=dbg['CFM'][:], in_=CFMa[:])
                    nc.sync.dma_start(out=dbg['HA0'][:], in_=HA[l][0:64])
                    nc.sync.dma_start(out=dbg['HB0'][:], in_=HB[l][0:64])
                    nc.sync.dma_start(out=dbg['RHT0'][:], in_=RHT0[:])
                    nc.sync.dma_start(out=dbg['RHT1'][:], in_=RHT1[:])
                    nc.sync.dma_start(out=dbg['HTW0'][:], in_=HLT0[:])
                    nc.sync.dma_start(out=dbg['HTW1'][:], in_=HLT1[:])
                cellno[0] += 1

            def cell_upper(p, l):
                gL, gLh = CT[p + 'gL'], CT[p + 'gLh']
                cLk0x, cLh = CT[p + 'cLk0x'], CT[p + 'cLh']
                cLx, cLrh = CT[p + 'cLx'], CT[p + 'cLrh']
                # -- gates: diffuse [h_{l-1} | h_l] (contiguous slot window) --
                Xg = wp3.tile([M0, BL, 400], dt.bfloat16, name='Xg', tag='Xg')
                # h_l(t-1) half is available since the previous step: pure
                # gap-filler work.  h_{l-1}(t) half is on the dependency chain.
                for b in range(0, BL, 2):
                    diffuse_half(l, Xg, slice(64, 128), b)
                for b in range(0, BL, 2):
                    diffuse_half(l - 1, Xg, slice(0, 64), b)
                RFMa = wp.tile([64, BL, 128], dt.bfloat16, name='RFMa', tag='RFMa')
                RFMb = wp2.tile([64, BL, NB], dt.bfloat16, name='RFMb', tag='RFMb')
                UFMa = wp.tile([64, BL, 128], dt.bfloat16, name='UFMa', tag='UFMa')
                UFMb = wp2.tile([64, BL, NB], dt.bfloat16, name='UFMb', tag='UFMb')
                bg = CT[p + 'bg'][:, l:l + 1]
                for ci, (n0, nw) in enumerate(NCH):
                    ps = gps.tile([M0, BL, nw], dt.float32, name='ops', tag='ops')
                    o = ps[:, :, :]
                    nc.tensor.matmul(o, gL[0:64, l - 1, 0, :], fm(HA[l - 1], HB[l - 1], ci), start=True, stop=False)
                    nc.tensor.matmul(o, gLh[:, l - 1, :], fm(HA[l], HB[l], ci), start=False, stop=False)
                    nc.tensor.matmul(o, gL[:, l - 1, 1, :], Xg[:, :, n0:n0 + nw], start=False, stop=False)
                    nc.tensor.matmul(o, gL[:, l - 1, 2, :], Xg[:, :, 200 + n0:200 + n0 + nw], start=False, stop=True)
                    rt_, sl = pr(RFMa, RFMb, ci)
                    ut_, _ = pr(UFMa, UFMb, ci)
                    nc.scalar.activation(rt_[:, :, sl], ps[0:64, :, :],
                                         AF.Sigmoid, bias=bg[0:64], scale=1.0)
                    nc.scalar.activation(ut_[:, :, sl], ps[64:128, :, :],
                                         AF.Sigmoid, bias=bg[64:128], scale=1.0)

                def cand_terms(ci, Xrh, RHa, RHb):
                    n0, nw = NCH[ci]
                    return [
                        (cLk0x[:, l - 1, :], fm(HA[l - 1], HB[l - 1], ci)),
                        (cLx[:, l - 1, 0, :], Xg[0:64, :, n0:n0 + nw]),
                        (cLx[:, l - 1, 1, :], Xg[0:64, :, 200 + n0:200 + n0 + nw]),
                        (cLh[:, l - 1, :], fm(RHa, RHb, ci)),
                        (cLrh[:, l - 1, 0, :], Xrh[0:64, :, n0:n0 + nw]),
                        (cLrh[:, l - 1, 1, :], Xrh[0:64, :, 200 + n0:200 + n0 + nw]),
                    ]

                rh_and_cand_tail(p, l, RFMa, RFMb, UFMa, UFMb, cand_terms)

            def cell0(p, x_terms, x0Ta, x0Tb, xfm0, xfm1):
                enc = (p == 'e')
                g0xa, g0xb, g0h = CT[p + 'g0x@a'], CT[p + 'g0x@b'], CT[p + 'g0h']
                c0xa, c0xb, c0h = CT[p + 'c0x@a'], CT[p + 'c0x@b'], CT[p + 'c0h']
                if x_terms:
                    Xga = wp3.tile([M0, BL, 400], dt.bfloat16, name='Xga', tag='Xg')
                    Xgb = wp.tile([M1, BL, 400], dt.bfloat16, name='Xgb', tag='Xgb')
                    for b in range(BL):
                        diffuse(128, [x0Ta[:, b, 0:128],
                                      x0Tb[0:M1, b, 0:128]], Xga[:, b, :], b)
                    for b in range(BL):
                        diffuse(M1, [x0Ta[:, b, 128:200],
                                     x0Tb[0:M1, b, 128:200]], Xgb[0:M1, b, :], b)
                Xgh = wp2.tile([64, BL, 400], dt.bfloat16, name='Xgh', tag='Xh')
                for b in range(0, BL, 2):
                    diffuse_pair([HLT0[:, 0, b:b + 2, :], HLT1[0:M1, 0, b:b + 2, :]],
                                 Xgh, b)
                if DBG and cellno[0] == 0:
                    nc.sync.dma_start(out=dbg['Xgh'][:], in_=Xgh[:])
                    if x_terms:
                        nc.sync.dma_start(out=dbg['Xga'][:], in_=Xga[:])
                        nc.sync.dma_start(out=dbg['Xgb'][:], in_=Xgb[0:M1])
                RFMa = wp.tile([64, BL, 128], dt.bfloat16, name='RFMa', tag='RFMa')
                RFMb = wp2.tile([64, BL, NB], dt.bfloat16, name='RFMb', tag='RFMb')
                UFMa = wp.tile([64, BL, 128], dt.bfloat16, name='UFMa', tag='UFMa')
                UFMb = wp2.tile([64, BL, NB], dt.bfloat16, name='UFMb', tag='UFMb')
                bg = CT[p + 'bg'][:, 0:1]
                for ci, (n0, nw) in enumerate(NCH):
                    ps = gps.tile([M0, BL, nw], dt.float32, name='ops', tag='ops')
                    o = ps[:, :, :]
                    first = True
                    if x_terms:
                        if enc:
                            nc.tensor.matmul(o, g0xa[:, 0, :], xfm0[:, :, n0:n0 + nw], start=True, stop=False)
                            nc.tensor.matmul(o, g0xb[0:M1, 0, :], xfm1[0:M1, :, n0:n0 + nw], start=False, stop=False)
                        else:
                            nc.tensor.matmul(o, CT['dWfg'][:], fm(HA[3], HB[3], ci), start=True, stop=False)
                        for k in (1, 2):
                            s = slice(200 * (k - 1) + n0, 200 * (k - 1) + n0 + nw)
                            nc.tensor.matmul(o, g0xa[:, k, :], Xga[:, :, s], start=False, stop=False)
                            nc.tensor.matmul(o, g0xb[0:M1, k, :], Xgb[0:M1, :, s], start=False, stop=False)
                        first = False
                    nc.tensor.matmul(o, g0h[:, 0, :], fm(HA[0], HB[0], ci), start=first, stop=False)
                    nc.tensor.matmul(o, g0h[:, 1, :], Xgh[0:64, :, n0:n0 + nw], start=False, stop=False)
                    nc.tensor.matmul(o, g0h[:, 2, :], Xgh[0:64, :, 200 + n0:200 + n0 + nw], start=False, stop=True)
                    rt_, sl = pr(RFMa, RFMb, ci)
                    ut_, _ = pr(UFMa, UFMb, ci)
                    nc.scalar.activation(rt_[:, :, sl], ps[0:64, :, :],
                                         AF.Sigmoid, bias=bg[0:64], scale=1.0)
                    nc.scalar.activation(ut_[:, :, sl], ps[64:128, :, :],
                                         AF.Sigmoid, bias=bg[64:128], scale=1.0)

                def cand_terms(ci, Xch, RHa, RHb):
                    n0, nw = NCH[ci]
                    terms = []
                    if x_terms:
                        if enc:
                            terms += [(c0xa[:, 0, :], xfm0[:, :, n0:n0 + nw]),
                                      (c0xb[0:M1, 0, :], xfm1[0:M1, :, n0:n0 + nw])]
                        else:
                            terms += [(CT['dWfc'][:], fm(HA[3], HB[3], ci))]
                        for k in (1, 2):
                            s = slice(200 * (k - 1) + n0, 200 * (k - 1) + n0 + nw)
                            terms += [(c0xa[:, k, :], Xga[:, :, s]),
                                      (c0xb[0:M1, k, :], Xgb[0:M1, :, s])]
                    terms += [(c0h[:, 0, :], fm(RHa, RHb, ci)),
                              (c0h[:, 1, :], Xch[0:64, :, n0:n0 + nw]),
                              (c0h[:, 2, :], Xch[0:64, :, 200 + n0:200 + n0 + nw])]
                    return terms

                rh_and_cand_tail(p, 0, RFMa, RFMb, UFMa, UFMb, cand_terms)

            # ---- encoder ----
            for t in range(enc_T):
                x0Ta = xp.tile([M0, BL, 200], dt.bfloat16, name='x0Ta', tag='x0Ta')
                x0Tb = xp.tile([M1, BL, 200], dt.bfloat16, name='x0Tb', tag='x0Tb')
                nc.sync.dma_start(out=x0Ta, in_=d['xTe'][t, 0])
                nc.sync.dma_start(out=x0Tb, in_=d['xTe'][t, 1, 0:M1])
                xfm0 = xp.tile([M0, BL, 200], dt.bfloat16, name='xfm0', tag='xfm0')
                xfm1 = xp.tile([M1, BL, 200], dt.bfloat16, name='xfm1', tag='xfm1')
                nc.sync.dma_start(out=xfm0, in_=d['xfme'][t, 0])
                nc.sync.dma_start(out=xfm1, in_=d['xfme'][t, 1, 0:M1])
                cell0('e', True, x0Ta, x0Tb, xfm0, xfm1)
                for l in range(1, L):
                    cell_upper('e', l)

            # ---- decoder ----
            x0Ta = x0Tb = None
            for t in range(dec_T):
                cell0('d', t > 0, x0Ta, x0Tb, None, None)
                for l in range(1, L):
                    cell_upper('d', l)
                pT = [wp2.tile([M0, BL, 200], dt.float32, name='pT0', tag='pT0'),
                      wp2.tile([M1, BL, 200], dt.float32, name='pT1', tag='pT1')]
                for mc, M, Hx in ((0, M0, None), (1, M1, None)):
                    for half in range(4):
                        pps = gps.tile([M0, 2, 200], dt.float32, name='ops', tag='ops')
                        for bb in range(2):
                            b = half * 2 + bb
                            if mc == 0:
                                lhsT = HA[3][0:65, b, 0:M0]
                            else:
                                lhsT = HB[3][0:65, b, 0:M1]
                            nc.tensor.matmul(pps[0:M, bb, :], lhsT, Wp[:],
                                             start=True, stop=True)
                        evac(half, pT[mc][0:M, half * 2:half * 2 + 2, :],
                             pps[0:M, :, :])
                nc.sync.dma_start(out=d['onm'][t, 0:M0], in_=pT[0][:])
                nc.sync.dma_start(out=d['onm'][t, M0:200], in_=pT[1][0:M1])
                if t < dec_T - 1:
                    x0Ta = xp.tile([M0, BL, 200], dt.bfloat16, name='x0Ta', tag='x0Ta')
                    x0Tb = xp.tile([M1, BL, 200], dt.bfloat16, name='x0Tb', tag='x0Tb')
                    nc.vector.tensor_copy(x0Ta[:], pT[0][:])
                    nc.vector.tensor_copy(x0Tb[:], pT[1][0:M1])

    nc.compile()
    return nc


# --------------------------------------------------------------------------
# host-side prep
# --------------------------------------------------------------------------

def _prep_shared(inputs):
    def bfc(x):
        return np.ascontiguousarray(np.asarray(x).astype(BF))

    S = np.asarray(inputs['support'], np.float64)
    S2 = 2.0 * (S @ S) - np.eye(N)
    SS = np.concatenate([S.astype(F32), S2.astype(F32)], axis=1)
    out = {
        'SS0': bfc(SS[0:M0]),
        'SS1': bfc(SS[M0:200]),
        'Wp': bfc(np.concatenate(
            [np.asarray(inputs['proj_W'], F32),
             np.asarray(inputs['proj_b'], F32)[None, :]], axis=0)),
    }
    for p, pre in (('e', 'enc_'), ('d', 'dec_')):
        Wg0 = np.asarray(inputs[pre + 'Wg0'], F32).reshape(264, 3, 128)
        Wc0 = np.asarray(inputs[pre + 'Wc0'], F32).reshape(264, 3, 64)
        out[p + 'g0x'] = bfc(Wg0[0:200])
        out[p + 'g0h'] = bfc(Wg0[200:264])
        out[p + 'c0x'] = bfc(Wc0[0:200])
        out[p + 'c0h'] = bfc(Wc0[200:264])
        WgL = np.asarray(inputs[pre + 'Wg'], F32).reshape(3, 128, 3, 128)
        WcL = np.asarray(inputs[pre + 'Wc'], F32).reshape(3, 128, 3, 64)
        out[p + 'gL'] = bfc(WgL.transpose(1, 0, 2, 3))          # (c, l-1, k, o)
        out[p + 'gLh'] = bfc(WgL[:, 64:128, 0, :].transpose(1, 0, 2))
        out[p + 'cLk0x'] = bfc(WcL[:, 0:64, 0, :].transpose(1, 0, 2))
        out[p + 'cLh'] = bfc(WcL[:, 64:128, 0, :].transpose(1, 0, 2))
        out[p + 'cLx'] = bfc(WcL[:, 0:64, 1:3, :].transpose(1, 0, 2, 3))
        out[p + 'cLrh'] = bfc(WcL[:, 64:128, 1:3, :].transpose(1, 0, 2, 3))
        bg = np.zeros((128, 4), F32)
        bc = np.zeros((64, 4), F32)
        bg[:, 0] = np.asarray(inputs[pre + 'bg0'], F32)
        bc[:, 0] = np.asarray(inputs[pre + 'bc0'], F32)
        bgl = np.asarray(inputs[pre + 'bg'], F32)
        bcl = np.asarray(inputs[pre + 'bc'], F32)
        for l in range(1, 4):
            bg[:, l] = bgl[l - 1]
            bc[:, l] = bcl[l - 1]
        if p == 'd':
            pb = np.asarray(inputs['proj_b'], np.float64)
            bg[:, 0] += (pb @ Wg0[0:200, 0, :].astype(np.float64)).astype(F32)
            bc[:, 0] += (pb @ Wc0[0:200, 0, :].astype(np.float64)).astype(F32)
            Wpf = np.asarray(inputs['proj_W'], np.float64)
            out['dWfg'] = bfc((Wpf @ Wg0[0:200, 0, :].astype(np.float64)).astype(F32))
            out['dWfc'] = bfc((Wpf @ Wc0[0:200, 0, :].astype(np.float64)).astype(F32))
        out[p + 'bg'] = np.ascontiguousarray(bg)
        out[p + 'bc'] = np.ascontiguousarray(bc)
    return out


def _prep_core_x(x_core, enc_T):
    x = np.asarray(x_core, F32).reshape(BL, -1, N, 200)[:, :enc_T]
    xb = x.astype(BF)
    xTe = np.zeros((enc_T, 2, M0, BL, 200), BF)
    xfme = np.zeros((enc_T, 2, M0, BL, 200), BF)
    xt = xb.transpose(1, 2, 0, 3)  # (T, n, b, f)
    xTe[:, 0, :, :, :] = xt[:, 0:M0]
    xTe[:, 1, 0:M1, :, :] = xt[:, M0:200]
    xf = xb.transpose(1, 3, 0, 2)  # (T, f, b, n)
    xfme[:, 0, :, :, :] = xf[:, 0:M0]
    xfme[:, 1, 0:M1, :, :] = xf[:, M0:200]
    return xTe, xfme


def get_program(enc_T=T, dec_T=T):
    key = (enc_T, dec_T)
    if key not in _CACHE:
        _CACHE[key] = _build(enc_T, dec_T)
    return _CACHE[key]


def make_in_maps(inputs, enc_T=T):
    shared = _prep_shared(inputs)
    x = np.asarray(inputs['inputs'], F32)
    in_maps = []
    for c in range(NCORES):
        xTe, xfme = _prep_core_x(x[c * BL:(c + 1) * BL], enc_T)
        m = dict(shared)
        m['xTe'] = xTe
        m['xfme'] = xfme
        in_maps.append(m)
    return in_maps


def assemble_output(results, dec_T=T):
    out = np.empty((B, dec_T, N * 200), F32)
    for c in range(NCORES):
        onm = results[c]['onm']
        out[c * BL:(c + 1) * BL] = (
            onm[:dec_T].transpose(2, 0, 1, 3).reshape(BL, dec_T, N * 200))
    return out


def kernel(**inputs):
    nc = get_program()
    in_maps = make_in_maps(inputs)
    res = run_bass_kernel_spmd(nc, in_maps, list(range(NCORES))).results
    return assemble_output(res)

